# revision 1
# baseline (speedup 1.0000x reference)
"""DGN agent (2-layer graph attention net) Trainium2 Bass kernel.

Strategy (pure data-parallel over batch, 8 cores):
  - Host prep: x transposed to feature-major [256, R]; mask expanded to
    block-diagonal C*mask^T tiles (4 batches -> one 128x128 group).
  - On chip, activations are kept feature-major [128 feat, rows-free]:
    every linear layer is one stationary-weight matmul per 512-col chunk.
  - Attention runs on 128-row "groups" (4 batches x 32 agents) as
    block-diagonal 128x128 matmuls. Masked softmax without max-subtraction
    (scores are O(10) for these inputs, verified): DVE computes
    t = (s + C) * mask_e (host-prepared block-diag 0/1 mask), ACT computes
    exp(t - C): valid -> exp(s); masked/off-diag -> exp(-C) == 0 exactly.
  - Softmax normalization: row-sums via DVE reduce, reciprocal, scale.
  - att @ v needs the contraction on the agent axis: v is produced
    row-major (lhsT = activations trick) and att is transposed with the
    DVE 32x32 stream-transpose (block-diagonal => in-place block
    transpose IS the full transpose).
"""

import numpy as np

import concourse.bass as bass
import concourse.mybir as mybir
import concourse.tile as tile
from concourse import bacc
from concourse.bass_utils import run_bass_kernel_spmd

F32 = mybir.dt.float32
AX = mybir.AxisListType
OP = mybir.AluOpType
AF = mybir.ActivationFunctionType

B, N, DIN, H, A = 4096, 32, 256, 128, 32
NCORES = 8
BC = B // NCORES          # batches per core
R = BC * N                # rows per core (16384)
MASK_C = 1024.0           # softmax offset; exp(s - C) == 0 for masked


def build_program(n_rows, rt=2048, n_cores=NCORES):
    """Emit the Bass/Tile program for one core processing n_rows rows."""
    assert n_rows % rt == 0 and rt % 512 == 0
    n_rt = n_rows // rt
    gpt = rt // 128           # attention groups per row tile
    ck = rt // 512            # 512-col chunks per row tile

    nc = bacc.Bacc("TRN2", target_bir_lowering=False, debug=False,
                   num_devices=n_cores)

    # ---- DRAM tensors -------------------------------------------------
    xt_d = nc.dram_tensor("xt", [DIN, n_rows], F32, kind="ExternalInput")
    met_d = nc.dram_tensor("met", [128, n_rows // 128, 128], F32,
                           kind="ExternalInput")  # block-diag 0/1 mask rows
    w_d = {}
    for nm in ("enc_w", "a1_vw", "a1_kw", "a1_qw", "a1_ow",
               "a2_vw", "a2_kw", "a2_qw", "a2_ow", "q_w"):
        shp = [DIN, H] if nm == "enc_w" else ([H, A] if nm == "q_w" else [H, H])
        w_d[nm] = nc.dram_tensor(nm, shp, F32, kind="ExternalInput")
    b_d = {}
    for nm in ("enc_b", "a1_vb", "a1_kb", "a1_qb", "a1_ob",
               "a2_vb", "a2_kb", "a2_qb", "a2_ob", "q_b"):
        b_d[nm] = nc.dram_tensor(nm, [A if nm == "q_b" else H], F32,
                                 kind="ExternalInput")
    y_d = nc.dram_tensor("y", [A, n_rows], F32, kind="ExternalOutput")

    with tile.TileContext(nc) as tc:
        with (
            tc.tile_pool(name="singles", bufs=1) as singles,
            tc.tile_pool(name="xt", bufs=2) as xt_pool,
            tc.tile_pool(name="met", bufs=2) as met_pool,
            tc.tile_pool(name="acts", bufs=3) as act_pool,
            tc.tile_pool(name="qk", bufs=2) as qk_pool,
            tc.tile_pool(name="sm", bufs=4) as sm_pool,
            tc.tile_pool(name="out", bufs=2) as out_pool,
            tc.tile_pool(name="pproj", bufs=2, space="PSUM") as pproj,
            tc.tile_pool(name="psc", bufs=2, space="PSUM") as psc,
            tc.tile_pool(name="pv", bufs=2, space="PSUM") as pv,
            tc.tile_pool(name="pao", bufs=2, space="PSUM") as pao,
        ):
            # ---- load constants once ---------------------------------
            w_sb = {}
            for nm, d in w_d.items():
                if nm == "enc_w":
                    t = singles.tile([128, 2, H], F32, tag=f"w_{nm}")
                    nc.sync.dma_start(out=t, in_=d.ap().rearrange(
                        "(c k) h -> k c h", c=2))
                else:
                    t = singles.tile([H, shp_free(nm)], F32, tag=f"w_{nm}")
                    nc.sync.dma_start(out=t, in_=d.ap())
                w_sb[nm] = t
            b_sb = {}
            for nm, d in b_d.items():
                n_el = A if nm == "q_b" else H
                t = singles.tile([n_el, 1], F32, tag=f"b_{nm}")
                nc.sync.dma_start(out=t, in_=d.ap().rearrange("(h o) -> h o", o=1))
                b_sb[nm] = t
            # v-biases as [1, 4*H] rows (tiled 4x) for one K=1 matmul per bank
            vb_row = {}
            for nm in ("a1_vb", "a2_vb"):
                t = singles.tile([1, 4, H], F32, tag=f"vr_{nm}")
                for rep in range(4):
                    nc.sync.dma_start(
                        out=t[:, rep, :],
                        in_=b_d[nm].ap().rearrange("(o h) -> o h", o=1))
                vb_row[nm] = t
            negc = singles.tile([128, 1], F32)
            nc.vector.memset(negc, -MASK_C)
            ones1 = singles.tile([1, H], F32)
            nc.vector.memset(ones1, 1.0)

            layers = [
                ("a1_vw", "a1_vb", "a1_kw", "a1_kb", "a1_qw", "a1_qb",
                 "a1_ow", "a1_ob"),
                ("a2_vw", "a2_vb", "a2_kw", "a2_kb", "a2_qw", "a2_qb",
                 "a2_ow", "a2_ob"),
            ]

            for irt in range(n_rt):
                r0 = irt * rt
                # ---- loads -------------------------------------------
                xt_sb = xt_pool.tile([128, 2, rt], F32, tag="xt")
                nc.sync.dma_start(
                    out=xt_sb,
                    in_=xt_d.ap().rearrange("(c k) r -> k c r", c=2)
                    [:, :, r0:r0 + rt])
                met_sb = met_pool.tile([128, gpt, 128], F32, tag="met")
                nc.sync.dma_start(
                    out=met_sb, in_=met_d.ap()[:, r0 // 128:r0 // 128 + gpt, :])

                # ---- encoder: h1 = relu(enc_w.T @ xT + enc_b) --------
                act = act_pool.tile([128, rt], F32, tag="act_in")
                for c in range(ck):
                    ps = pproj.tile([128, 512], F32, tag="proj")
                    sl = bass.ts(c, 512)
                    nc.tensor.matmul(ps, w_sb["enc_w"][:, 0, :],
                                     xt_sb[:, 0, sl], start=True, stop=False)
                    nc.tensor.matmul(ps, w_sb["enc_w"][:, 1, :],
                                     xt_sb[:, 1, sl], start=False, stop=True)
                    nc.scalar.activation(out=act[:, sl], in_=ps, func=AF.Relu,
                                         bias=b_sb["enc_b"], scale=1.0)

                for il, (vw, vb, kw, kb, qw, qb, ow, ob) in enumerate(layers):
                    # ---- q, k projections (feature-major) ------------
                    q_sb = qk_pool.tile([128, rt], F32, tag="q")
                    k_sb = qk_pool.tile([128, rt], F32, tag="k")
                    for c in range(ck):
                        sl = bass.ts(c, 512)
                        psq = pproj.tile([128, 512], F32, tag="proj")
                        nc.tensor.matmul(psq, w_sb[qw], act[:, sl],
                                         start=True, stop=True)
                        nc.scalar.activation(out=q_sb[:, sl], in_=psq,
                                             func=AF.Relu, bias=b_sb[qb],
                                             scale=1.0)
                        psk = pproj.tile([128, 512], F32, tag="proj")
                        nc.tensor.matmul(psk, w_sb[kw], act[:, sl],
                                         start=True, stop=True)
                        nc.vector.tensor_scalar(out=k_sb[:, sl], in0=psk,
                                                scalar1=b_sb[kb], scalar2=0.0,
                                                op0=OP.add, op1=OP.max)

                    ao_sb = act_pool.tile([128, rt], F32, tag="ao")
                    for gg in range(gpt // 4):     # 4 groups per psum bank
                        g0 = gg * 4
                        sc_ps = psc.tile([128, 4, 128], F32, tag="sc")
                        v_ps = pv.tile([128, 4, 128], F32, tag="v")
                        # v-bias broadcast first (start group), then per-group
                        nc.tensor.matmul(
                            v_ps.rearrange("p g w -> p (g w)"), ones1,
                            vb_row[vb].rearrange("p g w -> p (g w)"),
                            start=True, stop=False, skip_group_check=True)
                        for gi in range(4):
                            g = g0 + gi
                            gsl = bass.ts(g, 128)
                            nc.tensor.matmul(sc_ps[:, gi, :], q_sb[:, gsl],
                                             k_sb[:, gsl], start=True,
                                             stop=True)
                            # v row-major: lhsT = activations
                            nc.tensor.matmul(v_ps[:, gi, :], act[:, gsl],
                                             w_sb[vw], start=False, stop=True,
                                             skip_group_check=True)
                        # mask + offset on DVE: t = (s + C) * mask
                        t_sb = sm_pool.tile([128, 4, 128], F32, tag="t")
                        nc.vector.scalar_tensor_tensor(
                            out=t_sb, in0=sc_ps, scalar=MASK_C,
                            in1=met_sb[:, g0:g0 + 4, :],
                            op0=OP.add, op1=OP.mult)
                        e_sb = sm_pool.tile([128, 4, 128], F32, tag="e")
                        nc.scalar.activation(out=e_sb, in_=t_sb, func=AF.Exp,
                                             bias=negc, scale=1.0)
                        rs = sm_pool.tile([128, 4], F32, tag="rs")
                        nc.vector.reduce_sum(out=rs, in_=e_sb, axis=AX.X)
                        rr = sm_pool.tile([128, 4], F32, tag="rr")
                        nc.vector.reciprocal(out=rr, in_=rs)
                        att = sm_pool.tile([128, 4, 128], F32, tag="att")
                        for gi in range(4):
                            nc.gpsimd.tensor_scalar_mul(
                                out=att[:, gi, :], in0=e_sb[:, gi, :],
                                scalar1=rr[:, gi:gi + 1])
                        attT = sm_pool.tile([128, 512], F32, tag="attT")
                        nc.vector.transpose(
                            out=attT, in_=att.rearrange("p g w -> p (g w)"))
                        v_sb = sm_pool.tile([128, 4, 128], F32, tag="vsb")
                        nc.scalar.activation(out=v_sb, in_=v_ps, func=AF.Relu,
                                             scale=1.0)
                        ao_ps = pao.tile([128, 4, 128], F32, tag="ao")
                        attT3 = attT.rearrange("p (g w) -> p g w", g=4)
                        for gi in range(4):
                            nc.tensor.matmul(ao_ps[:, gi, :], v_sb[:, gi, :],
                                             attT3[:, gi, :], start=True,
                                             stop=True)
                        nc.vector.tensor_copy(
                            out=ao_sb[:, bass.ts(gg, 512)],
                            in_=ao_ps.rearrange("p g w -> p (g w)"))

                    # ---- out-projection: relu(ow.T @ ao + ob) --------
                    nact = act_pool.tile([128, rt], F32, tag="act_in")
                    for c in range(ck):
                        sl = bass.ts(c, 512)
                        ps = pproj.tile([128, 512], F32, tag="proj")
                        nc.tensor.matmul(ps, w_sb[ow], ao_sb[:, sl],
                                         start=True, stop=True)
                        nc.scalar.activation(out=nact[:, sl], in_=ps,
                                             func=AF.Relu, bias=b_sb[ob],
                                             scale=1.0)
                    act = nact

                # ---- final linear: q_w.T @ act + q_b ------------------
                o_sb = out_pool.tile([A, rt], F32, tag="o")
                for c in range(ck):
                    sl = bass.ts(c, 512)
                    ps = pproj.tile([128, 512], F32, tag="proj")
                    nc.tensor.matmul(ps[:A, :], w_sb["q_w"], act[:, sl],
                                     start=True, stop=True)
                    nc.vector.tensor_scalar(out=o_sb[:, sl], in0=ps[:A, :],
                                            scalar1=b_sb["q_b"], scalar2=None,
                                            op0=OP.add)
                nc.sync.dma_start(out=y_d.ap()[:, r0:r0 + rt], in_=o_sb)

    nc.compile()
    return nc


def shp_free(nm):
    return A if nm == "q_w" else H


def prep_inputs_core(x_c, mask_c):
    """Host-side prep for one core: x -> [DIN, R] fm; mask -> C*mask^T tiles."""
    rows = x_c.shape[0] * N
    xt = np.ascontiguousarray(
        x_c.reshape(rows, DIN).T).astype(np.float32)
    ng = x_c.shape[0] // 4
    m4 = mask_c.reshape(ng, 4, N, N)
    met = np.zeros((ng, 128, 128), np.float32)
    for b in range(4):
        met[:, 32 * b:32 * b + 32, 32 * b:32 * b + 32] = m4[:, b]
    # [G, m, n] -> [m, G, n] so each partition's DMA read is contiguous
    met = np.ascontiguousarray(met.transpose(1, 0, 2))
    return xt, met


_CACHE = {}
_PROFILE = False


def build_in_maps(inputs):
    inputs = {k: np.asarray(v, dtype=np.float32) for k, v in inputs.items()}
    x, mask = inputs["x"], inputs["mask"]
    wnames = ("enc_w", "a1_vw", "a1_kw", "a1_qw", "a1_ow",
              "a2_vw", "a2_kw", "a2_qw", "a2_ow", "q_w")
    bnames = ("enc_b", "a1_vb", "a1_kb", "a1_qb", "a1_ob",
              "a2_vb", "a2_kb", "a2_qb", "a2_ob", "q_b")
    in_maps = []
    for c in range(NCORES):
        xt, met = prep_inputs_core(x[c * BC:(c + 1) * BC],
                                   mask[c * BC:(c + 1) * BC])
        m = {"xt": xt, "met": met}
        for nm in wnames + bnames:
            m[nm] = inputs[nm]
        in_maps.append(m)
    return in_maps


def kernel(**inputs):
    if "nc" not in _CACHE:
        _CACHE["nc"] = build_program(R)
    res = run_bass_kernel_spmd(_CACHE["nc"], build_in_maps(inputs),
                               core_ids=list(range(NCORES)))
    outs = [r["y"].T.reshape(BC, N, A) for r in res.results]
    return np.concatenate(outs, axis=0)



# revision 15
# speedup vs baseline: 2.1799x; 2.1799x over previous
"""DGN agent (2-layer graph attention net) Trainium2 Bass kernel.

V2 design (pure data-parallel over batch, 8 cores), all-bf16 matmuls:
  - Host prep (bf16): x transposed feature-major [256, R]; mask packed as
    additive offsets moff[32*(b%4)+i, 32*(b//4)+j] = -1024 if masked else 0
    (dense "pk" layout: every element meaningful, 4x less on-chip work than
    block-diagonal, and exp(s-1024)==0 exactly kills masked edges).
  - Activations feature-major [128, rt] bf16; projections are stationary
    weight matmuls (bf16 = 1 cy/row, 4x faster than fp32).
  - Scores: per batch b one 32-free matmul q_b^T k_b written to the pk psum
    block; the mask offset is pre-accumulated into psum with ONE identity
    matmul per tile-layer (no vector-engine mask op at all).
  - Softmax on the dense pk tile: exp (ACT, psum->bf16), segmented row-sum
    (DVE, [128,16,32]->[128,16]), reciprocal, broadcast multiply, 32x32
    stream transpose -> attT in pk layout.
  - o-projection folded into v-path: vo = relu(v) @ ow computed row-major
    per 128-row group; then att@vo directly yields the NEXT layer's
    pre-activation feature-major in psum (no separate o-proj matmul, no
    ao copy; v bias becomes a free per-partition ACT bias).
  - Elementwise spread across ACT (enc/v/nact relu, exp), DVE (k relu,
    reduce, recip, scale, transpose, final bias), Pool (q relu, vo copy).
"""

import numpy as np

import concourse.bass as bass
import concourse.mybir as mybir
import concourse.tile as tile
from concourse import bacc
from concourse.bass_utils import run_bass_kernel_spmd

F32 = mybir.dt.float32
BF16 = mybir.dt.bfloat16
AX = mybir.AxisListType
OP = mybir.AluOpType
AF = mybir.ActivationFunctionType

B, N, DIN, H, A = 4096, 32, 256, 128, 32
NCORES = 8
BC = B // NCORES          # batches per core (512)
R = BC * N                # rows per core (16384)
MASK_C = 1024.0


def build_program(n_rows, rt=2048, n_cores=NCORES):
    """Emit the Bass/Tile program for one core processing n_rows rows."""
    assert n_rows % rt == 0 and rt % 512 == 0
    n_rt = n_rows // rt
    nb = rt // 32             # batches per row tile
    ck = rt // 512            # 512-col chunks per row tile
    pkw = nb * 8              # pk tile width = nb/4 * 32

    nc = bacc.Bacc("TRN2", target_bir_lowering=False, debug=False,
                   num_devices=n_cores)

    # ---- DRAM tensors -------------------------------------------------
    xt_d = nc.dram_tensor("xt", [DIN, n_rows], BF16, kind="ExternalInput")
    moff_d = nc.dram_tensor("moff", [128, n_rows // 128, 128], BF16,
                            kind="ExternalInput")
    id_d = nc.dram_tensor("ident", [128, 128], BF16, kind="ExternalInput")
    w_d = {}
    for nm in ("enc_w", "a1_vw", "a1_kw", "a1_qw", "a1_ow",
               "a2_vw", "a2_kw", "a2_qw", "a2_ow", "q_w"):
        shp = [DIN, H] if nm == "enc_w" else ([H, A] if nm == "q_w" else [H, H])
        w_d[nm] = nc.dram_tensor(nm, shp, BF16, kind="ExternalInput")
    b_d = {}
    for nm in ("enc_b", "a1_vb", "a1_kb", "a1_qb", "a1_ob",
               "a2_vb", "a2_kb", "a2_qb", "a2_ob", "q_b"):
        b_d[nm] = nc.dram_tensor(nm, [A if nm == "q_b" else H], F32,
                                 kind="ExternalInput")
    y_d = nc.dram_tensor("y", [A, n_rows], F32, kind="ExternalOutput")

    with tile.TileContext(nc) as tc:
        with (
            tc.tile_pool(name="singles", bufs=1) as singles,
            tc.tile_pool(name="xt", bufs=2) as xt_pool,
            tc.tile_pool(name="moff", bufs=2) as moff_pool,
            tc.tile_pool(name="acts", bufs=3) as act_pool,
            tc.tile_pool(name="qkv", bufs=2) as qkv_pool,
            tc.tile_pool(name="vo", bufs=2) as vo_pool,
            tc.tile_pool(name="sm", bufs=2) as sm_pool,
            tc.tile_pool(name="att", bufs=5) as attT_pool,
            tc.tile_pool(name="out", bufs=2) as out_pool,
            tc.tile_pool(name="pproj", bufs=3, space="PSUM") as pproj,
            tc.tile_pool(name="psc", bufs=1, space="PSUM") as psc,
            tc.tile_pool(name="pvo", bufs=1, space="PSUM") as pvo,
            tc.tile_pool(name="pna", bufs=2, space="PSUM") as pna,
        ):
            # ---- load constants once ---------------------------------
            w_sb = {}
            for nm, d in w_d.items():
                if nm == "enc_w":
                    t = singles.tile([128, 2, H], BF16, tag=f"w_{nm}")
                    nc.sync.dma_start(out=t, in_=d.ap().rearrange(
                        "(c k) h -> k c h", c=2))
                else:
                    t = singles.tile([H, A if nm == "q_w" else H], BF16,
                                     tag=f"w_{nm}")
                    nc.sync.dma_start(out=t, in_=d.ap())
                w_sb[nm] = t
            b_sb = {}
            for nm, d in b_d.items():
                n_el = A if nm == "q_b" else H
                t = singles.tile([n_el, 1], F32, tag=f"b_{nm}")
                nc.sync.dma_start(out=t, in_=d.ap().rearrange("(h o) -> h o", o=1))
                b_sb[nm] = t
            ident = singles.tile([128, 128], BF16, tag="ident")
            nc.sync.dma_start(out=ident, in_=id_d.ap())

            layers = [
                ("a1_vw", "a1_vb", "a1_kw", "a1_kb", "a1_qw", "a1_qb",
                 "a1_ow", "a1_ob"),
                ("a2_vw", "a2_vb", "a2_kw", "a2_kb", "a2_qw", "a2_qb",
                 "a2_ow", "a2_ob"),
            ]

            for irt in range(n_rt):
                r0 = irt * rt
                # ---- loads -------------------------------------------
                xt_sb = xt_pool.tile([128, 2, rt], BF16, tag="xt")
                nc.sync.dma_start(
                    out=xt_sb,
                    in_=xt_d.ap().rearrange("(c k) r -> k c r", c=2)
                    [:, :, r0:r0 + rt])
                gpt = rt // 128
                moff_sb = moff_pool.tile([128, gpt, 128], BF16, tag="moff")
                nc.sync.dma_start(
                    out=moff_sb,
                    in_=moff_d.ap()[:, irt * gpt:(irt + 1) * gpt, :])

                # ---- encoder: h1 = relu(enc_w.T @ xT + enc_b) --------
                act = act_pool.tile([128, rt], BF16, tag="act")
                for c in range(ck):
                    ps = pproj.tile([128, 512], F32, tag="proj")
                    sl = bass.ts(c, 512)
                    nc.tensor.matmul(ps, w_sb["enc_w"][:, 0, :],
                                     xt_sb[:, 0, sl], start=True, stop=False)
                    nc.tensor.matmul(ps, w_sb["enc_w"][:, 1, :],
                                     xt_sb[:, 1, sl], start=False, stop=True)
                    nc.scalar.activation(out=act[:, sl], in_=ps, func=AF.Relu,
                                         bias=b_sb["enc_b"], scale=1.0)

                for il, (vw, vb, kw, kb, qw, qb, ow, ob) in enumerate(layers):
                    # ---- q, k, v projections (feature-major) ---------
                    q_sb = qkv_pool.tile([128, rt], BF16, tag="q")
                    k_sb = qkv_pool.tile([128, rt], BF16, tag="k")
                    v_sb = qkv_pool.tile([128, rt], BF16, tag="v")
                    for c in range(ck):
                        sl = bass.ts(c, 512)
                        psq = pproj.tile([128, 512], F32, tag="proj")
                        nc.tensor.matmul(psq, w_sb[qw], act[:, sl],
                                         start=True, stop=True)
                        # q-relu on DVE (gpsimd cannot read PSUM)
                        nc.vector.tensor_scalar(out=q_sb[:, sl], in0=psq,
                                                scalar1=b_sb[qb], scalar2=0.0,
                                                op0=OP.add, op1=OP.max)
                        psk = pproj.tile([128, 512], F32, tag="proj")
                        nc.tensor.matmul(psk, w_sb[kw], act[:, sl],
                                         start=True, stop=True)
                        # k-relu on DVE
                        nc.vector.tensor_scalar(out=k_sb[:, sl], in0=psk,
                                                scalar1=b_sb[kb], scalar2=0.0,
                                                op0=OP.add, op1=OP.max)
                        psv = pproj.tile([128, 512], F32, tag="proj")
                        nc.tensor.matmul(psv, w_sb[vw], act[:, sl],
                                         start=True, stop=True)
                        # v-relu on ACT (bias is per-partition: free)
                        nc.scalar.activation(out=v_sb[:, sl], in_=psv,
                                             func=AF.Relu, bias=b_sb[vb],
                                             scale=1.0)

                    # ---- scores block-diag + additive mask (safe) ----
                    ao_done = []
                    for gg in range(rt // 512):
                        sc_ps = psc.tile([128, 4, 128], F32, tag="sc")
                        nc.tensor.matmul(
                            sc_ps.rearrange("p g w -> p (g w)"), ident,
                            moff_sb[:, gg * 4:gg * 4 + 4, :].rearrange(
                                "p g w -> p (g w)"),
                            start=True, stop=False, skip_group_check=True)
                        for gi in range(4):
                            g4 = gg * 4 + gi
                            gsl = bass.ts(g4, 128)
                            nc.tensor.matmul(sc_ps[:, gi, :], q_sb[:, gsl],
                                             k_sb[:, gsl], start=False,
                                             stop=True, skip_group_check=True)
                        e_sb = sm_pool.tile([128, 4, 128], BF16, tag="e")
                        nc.scalar.activation(out=e_sb, in_=sc_ps, func=AF.Exp,
                                             scale=1.0)
                        rs = sm_pool.tile([128, 4], F32, tag="rs")
                        nc.vector.reduce_sum(out=rs, in_=e_sb, axis=AX.X)
                        rr = sm_pool.tile([128, 4], F32, tag="rr")
                        nc.vector.reciprocal(out=rr, in_=rs)
                        att = sm_pool.tile([128, 4, 128], BF16, tag="att")
                        for gi in range(4):
                            nc.gpsimd.tensor_scalar_mul(
                                out=att[:, gi, :], in0=e_sb[:, gi, :],
                                scalar1=rr[:, gi:gi + 1])
                        attT = attT_pool.tile([128, 4, 128], BF16, tag="attT")
                        nc.vector.transpose(
                            out=attT.rearrange("p g w -> p (g w)"),
                            in_=att.rearrange("p g w -> p (g w)"))
                        ao_done.append(attT)

                    # ---- vo = v @ ow, row-major per 128-row group ----
                    vgc = min(8, rt // 128)     # vo groups per psum tile
                    vo_sb = vo_pool.tile([128, rt // 128, 128], BF16, tag="vo")
                    for gg in range(rt // (128 * vgc)):
                        vo_ps = pvo.tile([128, vgc, 128], F32, tag="vo")
                        for gi in range(vgc):
                            g4 = gg * vgc + gi
                            nc.tensor.matmul(
                                vo_ps[:, gi, :],
                                v_sb[:, g4 * 128:(g4 + 1) * 128],
                                w_sb[ow], start=True, stop=True)
                        nc.scalar.activation(
                            out=vo_sb[:, gg * vgc:(gg + 1) * vgc, :],
                            in_=vo_ps, func=AF.Copy, scale=1.0)

                    # ---- att @ vo -> next act (feature-major psum) ---
                    nact = act_pool.tile([128, rt], BF16, tag="act")
                    for gg in range(rt // 512):
                        na_ps = pna.tile([128, 4, 128], F32, tag="na")
                        attT = ao_done[gg]
                        for gi in range(4):
                            g4 = gg * 4 + gi
                            nc.tensor.matmul(
                                na_ps[:, gi, :],
                                vo_sb[:, g4, :],
                                attT[:, gi, :],
                                start=True, stop=True)
                        nc.scalar.activation(
                            out=nact[:, bass.ts(gg, 512)], in_=na_ps,
                            func=AF.Relu, bias=b_sb[ob], scale=1.0)
                    act = nact

                # ---- final linear: q_w.T @ act + q_b ------------------
                o_sb = out_pool.tile([A, rt], F32, tag="o")
                for c in range(ck):
                    sl = bass.ts(c, 512)
                    ps = pproj.tile([128, 512], F32, tag="proj")
                    nc.tensor.matmul(ps[:A, :], w_sb["q_w"], act[:, sl],
                                     start=True, stop=True)
                    nc.vector.tensor_scalar(out=o_sb[:, sl], in0=ps[:A, :],
                                            scalar1=b_sb["q_b"], scalar2=None,
                                            op0=OP.add)
                nc.sync.dma_start(out=y_d.ap()[:, r0:r0 + rt], in_=o_sb)

    nc.compile()
    return nc


def prep_inputs_core(x_c, mask_c):
    """Host prep for one core: x -> [DIN, R] bf16; mask -> block-diag
    additive offsets moff [128, R//128, 128] bf16 (0 valid, -C elsewhere)."""
    import ml_dtypes
    bf16 = ml_dtypes.bfloat16
    rows = x_c.shape[0] * N
    xt = np.ascontiguousarray(
        x_c.reshape(rows, DIN).T).astype(bf16)
    ng = x_c.shape[0] // 4
    m4 = mask_c.reshape(ng, 4, N, N)
    met = np.full((ng, 128, 128), -MASK_C, np.float32)
    for b in range(4):
        met[:, 32 * b:32 * b + 32, 32 * b:32 * b + 32] = \
            (m4[:, b] - 1.0) * MASK_C
    met = np.ascontiguousarray(met.transpose(1, 0, 2)).astype(bf16)
    return xt, met


_CACHE = {}


def build_in_maps(inputs):
    import ml_dtypes
    bf16 = ml_dtypes.bfloat16
    inputs = {k: np.asarray(v, dtype=np.float32) for k, v in inputs.items()}
    x, mask = inputs["x"], inputs["mask"]
    wnames = ("enc_w", "a1_vw", "a1_kw", "a1_qw", "a1_ow",
              "a2_vw", "a2_kw", "a2_qw", "a2_ow", "q_w")
    bnames = ("enc_b", "a1_vb", "a1_kb", "a1_qb", "a1_ob",
              "a2_vb", "a2_kb", "a2_qb", "a2_ob", "q_b")
    ident = np.eye(128, dtype=bf16)
    w_bf = {nm: inputs[nm].astype(bf16) for nm in wnames}
    in_maps = []
    for c in range(NCORES):
        xt, moff = prep_inputs_core(x[c * BC:(c + 1) * BC],
                                    mask[c * BC:(c + 1) * BC])
        m = {"xt": xt, "moff": moff, "ident": ident}
        for nm in wnames:
            m[nm] = w_bf[nm]
        for nm in bnames:
            m[nm] = inputs[nm]
        in_maps.append(m)
    return in_maps


def kernel(**inputs):
    if "nc" not in _CACHE:
        _CACHE["nc"] = build_program(R)
    res = run_bass_kernel_spmd(_CACHE["nc"], build_in_maps(inputs),
                               core_ids=list(range(NCORES)))
    outs = [np.asarray(r["y"], dtype=np.float32).T.reshape(BC, N, A)
            for r in res.results]
    return np.concatenate(outs, axis=0)
